# revision 1
# baseline (speedup 1.0000x reference)
"""2-layer GCN on 8 Trainium2 cores via Bass/Tile.

Strategy:
- Nodes sharded across cores (contiguous blocks of SHARD rows each).
- Per layer: each core computes xw = x_shard @ W locally (node-major),
  AllGather -> full table [N_PAD, HID] in DRAM of every core.
- Edges are bucketed on host by (dest core, dest block of 128, src chunk)
  where src chunks are <=32768 rows (int16 gather index limit). Every
  bucket is padded to a uniform size B (multiple of 128) so all 8 cores
  run the same instruction stream.
- Aggregation per dest block: dma_gather of the per-edge source rows
  (512B each), on-chip one-hot matrix S[e, d] = (dest_local[e]==d)*norm[e]
  built with one tensor_scalar(is_equal, mult) from an IOTA tile, then
  PE matmul gathered.T @ S accumulated in PSUM => agg.T [feat, dest].
- tanh(agg + b) on ACT (bias is per-partition in feature-major layout).
- Layer 1 output goes through a second matmul with W2 to produce the
  layer-2 gather table (node-major), AllGather #2, then layer 2 repeats
  aggregation; final h2.T written feature-major, host transposes.
"""
import numpy as np
import concourse.bass as bass
import concourse.bacc as bacc
import concourse.tile as tile
import concourse.mybir as mybir

F32 = mybir.dt.float32
I16 = mybir.dt.int16
AF = mybir.ActivationFunctionType
ALU = mybir.AluOpType


class Cfg:
    def __init__(self, n_nodes, in_dim, hid, n_cores=8, blocks_per_sb=4,
                 chunk_rows=25088):
        self.n_nodes = n_nodes
        self.in_dim = in_dim
        self.hid = hid
        self.n_cores = n_cores
        self.bpsb = blocks_per_sb
        # shard = nodes per core, multiple of 128
        shard = -(-n_nodes // n_cores)
        shard = -(-shard // 128) * 128
        self.shard = shard
        self.n_pad = shard * n_cores
        self.blocks = shard // 128          # dest blocks per core
        self.chunk_rows = chunk_rows        # must divide n_pad and be <= 32767
        assert self.n_pad % chunk_rows == 0
        self.n_chunks = self.n_pad // chunk_rows
        assert chunk_rows <= 32767
        self.n_sb = -(-self.blocks // self.bpsb)
        self.row_bytes = hid * 4


def preprocess(edge_index, cfg: Cfg):
    """Bucket, pad, and lay out edges. Returns per-core meta arrays and B."""
    n, npad = cfg.n_nodes, cfg.n_pad
    row = np.asarray(edge_index[0], dtype=np.int64)
    col = np.asarray(edge_index[1], dtype=np.int64)
    loops = np.arange(n, dtype=np.int64)
    row = np.concatenate([row, loops])
    col = np.concatenate([col, loops])
    deg = np.bincount(col, minlength=n).astype(np.float32)
    dinv = np.where(deg > 0, 1.0 / np.sqrt(deg), 0.0).astype(np.float32)
    norm = (dinv[row] * dinv[col]).astype(np.float32)

    core = col // cfg.shard
    block = (col % cfg.shard) // 128
    dest_local = (col % 128).astype(np.float32)
    chunk = row // cfg.chunk_rows
    src_rel = (row % cfg.chunk_rows).astype(np.int16)

    # bucket id = ((core*blocks + block)*n_chunks + chunk)
    nbuck = cfg.n_cores * cfg.blocks * cfg.n_chunks
    bid = (core * cfg.blocks + block) * cfg.n_chunks + chunk
    order = np.argsort(bid, kind="stable")
    bid_s = bid[order]
    counts = np.bincount(bid_s, minlength=nbuck)
    # per-(block, chunk) padded size: max over cores, rounded up to 128
    cmax = counts.reshape(cfg.n_cores, cfg.blocks, cfg.n_chunks).max(axis=0)
    Bbc = (-(-cmax // 128) * 128).astype(np.int64)  # [blocks, n_chunks]

    # bucket start offsets in sorted arrays
    starts = np.zeros(nbuck + 1, dtype=np.int64)
    np.cumsum(counts, out=starts[1:])

    src_rel_s = src_rel[order]
    dest_local_s = dest_local[order]
    norm_s = norm[order]

    per_core = []
    for c in range(cfg.n_cores):
        # padded layout: for sb, chunk: concat blocks of sb, each to Bbc[b,ch]
        sb_sizes = []
        idx_parts = []
        dest_parts = []
        norm_parts = []
        for sb in range(cfg.n_sb):
            b0 = sb * cfg.bpsb
            nb = min(cfg.bpsb, cfg.blocks - b0)
            sb_sizes.append(nb)
            for ch in range(cfg.n_chunks):
                for b in range(b0, b0 + nb):
                    k = (c * cfg.blocks + b) * cfg.n_chunks + ch
                    s, e = starts[k], starts[k + 1]
                    cnt = e - s
                    Bp = Bbc[b, ch]
                    ip = np.zeros(Bp, dtype=np.int16)
                    dp = np.full(Bp, 128.0, dtype=np.float32)
                    np_ = np.zeros(Bp, dtype=np.float32)
                    ip[:cnt] = src_rel_s[s:e]
                    dp[:cnt] = dest_local_s[s:e]
                    np_[:cnt] = norm_s[s:e]
                    idx_parts.append(ip)
                    dest_parts.append(dp)
                    norm_parts.append(np_)
        idx_flat = np.concatenate(idx_parts)
        dest_flat = np.concatenate(dest_parts)
        norm_flat = np.concatenate(norm_parts)
        # wrap idx to [ntot/16, 16] -> [16, ntot/16]; replicate to 128 parts.
        ntot = idx_flat.size
        wrapped = idx_flat.reshape(ntot // 16, 16).T  # [16, ntot/16]
        wrapped = np.tile(wrapped, (8, 1))            # [128, ntot/16]
        # dest/norm to [128, ntot/128]: edge j -> [j%128, j//128]
        dest_w = dest_flat.reshape(ntot // 128, 128).T.copy()
        norm_w = norm_flat.reshape(ntot // 128, 128).T.copy()
        per_core.append(dict(idx=wrapped, dest=dest_w, norm=norm_w))
    return per_core, Bbc, sb_sizes


def build_kernel(cfg: Cfg, Bbc, sb_sizes, nqueues=4, repeat=1, timing_loop=0):
    """Build the SPMD Bass program (same for all cores).

    Bbc: [blocks, n_chunks] per-bucket padded sizes (multiples of 128).
    repeat: duplicate the whole compute pipeline (for HW timing by
    wall-clock differencing); output is from the last copy.
    timing_loop: if >0, wrap the compute (minus collectives) in a
    device-side For_i loop of that count — OUTPUT IS GARBAGE, timing only.
    """
    hid, ind = cfg.hid, cfg.in_dim
    Bbc = np.asarray(Bbc, dtype=np.int64)
    n_idx_tot = int(Bbc.sum())
    nc = bacc.Bacc("TRN2", target_bir_lowering=False, debug=False,
                   num_devices=cfg.n_cores, num_swdge_queues=nqueues)

    # ---- I/O ----
    x_t = nc.dram_tensor("x_t", [ind, cfg.shard], F32, kind="ExternalInput")
    w1 = nc.dram_tensor("w1", [ind, hid], F32, kind="ExternalInput")
    w2 = nc.dram_tensor("w2", [hid, hid], F32, kind="ExternalInput")
    b1 = nc.dram_tensor("b1", [hid, 1], F32, kind="ExternalInput")
    b2 = nc.dram_tensor("b2", [hid, 1], F32, kind="ExternalInput")
    iota_in = nc.dram_tensor("iota", [128, 128], F32, kind="ExternalInput")
    idx_in = nc.dram_tensor("idx", [128, n_idx_tot // 16], I16, kind="ExternalInput")
    dest_in = nc.dram_tensor("dest", [128, n_idx_tot // 128], F32, kind="ExternalInput")
    norm_in = nc.dram_tensor("norm", [128, n_idx_tot // 128], F32, kind="ExternalInput")
    out = nc.dram_tensor("out", [hid, cfg.shard], F32, kind="ExternalOutput")

    rg = [list(range(cfg.n_cores))]

    with tile.TileContext(nc) as tc:
        with (
            tc.tile_pool(name="dram", bufs=1, space="DRAM") as dram,
            tc.tile_pool(name="const", bufs=1) as cpool,
            tc.tile_pool(name="xin", bufs=4) as xpool,
            tc.tile_pool(name="mmps", bufs=2, space="PSUM") as mmps,
            tc.tile_pool(name="aggps", bufs=3, space="PSUM") as aggps,
            tc.tile_pool(name="gat", bufs=2 * cfg.n_chunks) as gatpool,
            tc.tile_pool(name="meta", bufs=8) as metapool,
            tc.tile_pool(name="sgen", bufs=4) as spool,
            tc.tile_pool(name="hst", bufs=4) as hpool,
            tc.tile_pool(name="wr", bufs=4) as wrpool,
        ):
            shard1 = dram.tile([cfg.shard, hid], F32)
            shard2 = dram.tile([cfg.shard, hid], F32)
            table1 = dram.tile([cfg.n_pad, hid], F32)
            table2 = dram.tile([cfg.n_pad, hid], F32)
            # constants
            iota_t = cpool.tile([128, 128], F32)
            nc.sync.dma_start(iota_t[:], iota_in[:])
            kparts = ind // 128
            w1_t = cpool.tile([128, kparts, hid], F32, tag="w1")
            nc.sync.dma_start(w1_t[:], w1[:].rearrange("(k p) h -> p k h", p=128))
            w2_t = cpool.tile([128, hid], F32, tag="w2")
            nc.sync.dma_start(w2_t[:], w2[:])
            b1_t = cpool.tile([128, 1], F32, tag="b1")
            nc.sync.dma_start(b1_t[:], b1[:])
            b2_t = cpool.tile([128, 1], F32, tag="b2")
            nc.sync.dma_start(b2_t[:], b2[:])

            # precompute meta offsets: layout is for sb: for ch: for b in sb
            seg_off = {}  # (sb, ch) -> start offset in edge units
            seg_len = {}  # (sb, ch) -> total edges in segment
            _o = 0
            for sb, nb in enumerate(sb_sizes):
                b0 = sb * cfg.bpsb
                for ch in range(cfg.n_chunks):
                    seg_off[(sb, ch)] = _o
                    L = int(Bbc[b0:b0 + nb, ch].sum())
                    seg_len[(sb, ch)] = L
                    _o += L
            assert _o == n_idx_tot

            if timing_loop:
                # Pre-fill tables once (garbage-safe shapes), keep collectives
                # out of the loop.
                nc.gpsimd.collective_compute(
                    "AllGather", ALU.bypass, replica_groups=rg,
                    ins=[shard1.opt()], outs=[table1.opt()])
                nc.gpsimd.collective_compute(
                    "AllGather", ALU.bypass, replica_groups=rg,
                    ins=[shard2.opt()], outs=[table2.opt()])
                loop_cm = tc.For_i(0, timing_loop, 1)
                loop_cm.__enter__()

            for _rep in range(repeat):
                # ---- prologue: xw1 = x @ W1 (node-major) -> shard1 ----
                for n in range(cfg.blocks):
                    xt = xpool.tile([128, kparts, 128], F32)
                    nc.sync.dma_start(
                        xt[:], x_t[:, n * 128:(n + 1) * 128].rearrange("(k p) d -> p k d", p=128))
                    ps = mmps.tile([128, hid], F32)
                    for k in range(kparts):
                        nc.tensor.matmul(ps[:], xt[:, k, :], w1_t[:, k, :],
                                         start=(k == 0), stop=(k == kparts - 1))
                    sb_ = wrpool.tile([128, hid], F32)
                    nc.vector.tensor_copy(sb_[:], ps[:])
                    nc.sync.dma_start(shard1[n * 128:(n + 1) * 128, :], sb_[:])

                if not timing_loop:
                    nc.gpsimd.collective_compute(
                        "AllGather", ALU.bypass, replica_groups=rg,
                        ins=[shard1.opt()], outs=[table1.opt()])

                # ---- layers ----
                for layer in (1, 2):
                    table = table1 if layer == 1 else table2
                    bias_t = b1_t if layer == 1 else b2_t
                    blk_base = 0
                    for sb, nb in enumerate(sb_sizes):
                        b0 = sb * cfg.bpsb
                        gts = []
                        for ch in range(cfg.n_chunks):
                            o = seg_off[(sb, ch)]
                            L = seg_len[(sb, ch)]
                            if L == 0:
                                gts.append((None, None, None))
                                continue
                            it = metapool.tile([128, L // 16], I16, tag="it")
                            nc.sync.dma_start(it[:], idx_in[:, o // 16:(o + L) // 16])
                            dt_ = metapool.tile([128, L // 128], F32, tag="dt")
                            nc.sync.dma_start(dt_[:], dest_in[:, o // 128:(o + L) // 128])
                            nt = metapool.tile([128, L // 128], F32, tag="nt")
                            nc.sync.dma_start(nt[:], norm_in[:, o // 128:(o + L) // 128])
                            gt = gatpool.tile([128, L // 128, hid], F32, tag="gt")
                            nc.gpsimd.dma_gather(
                                gt[:],
                                table[ch * cfg.chunk_rows:(ch + 1) * cfg.chunk_rows, :],
                                it[:], L, L, hid,
                                single_packet=False, queue_num=ch % nqueues)
                            gts.append((gt, dt_, nt))
                        # per dest block: accumulate over (chunk, group) sequentially
                        for b in range(nb):
                            agg = aggps.tile([128, 128], F32, tag="agg")
                            n_mm = sum(int(Bbc[b0 + b, ch]) // 128 for ch in range(cfg.n_chunks))
                            mm_i = 0
                            for ch, (gt, dt_, nt) in enumerate(gts):
                                gpb = int(Bbc[b0 + b, ch]) // 128
                                goff = int(Bbc[b0:b0 + b, ch].sum()) // 128
                                for k in range(gpb):
                                    g = goff + k
                                    s_t = spool.tile([128, 128], F32)
                                    nc.vector.tensor_scalar(
                                        s_t[:], iota_t[:],
                                        dt_[:, g:g + 1], nt[:, g:g + 1],
                                        ALU.is_equal, ALU.mult)
                                    nc.tensor.matmul(
                                        agg[:], gt[:, g, :], s_t[:],
                                        start=(mm_i == 0),
                                        stop=(mm_i == n_mm - 1))
                                    mm_i += 1
                            blk = blk_base + b
                            h_t = hpool.tile([128, 128], F32)
                            nc.scalar.activation(h_t[:], agg[:], AF.Tanh, bias=bias_t[:])
                            if layer == 1:
                                ps2 = mmps.tile([128, hid], F32, tag="ps2")
                                nc.tensor.matmul(ps2[:], h_t[:], w2_t[:], start=True, stop=True)
                                hw = wrpool.tile([128, hid], F32, tag="hw")
                                nc.vector.tensor_copy(hw[:], ps2[:])
                                nc.sync.dma_start(shard2[blk * 128:(blk + 1) * 128, :], hw[:])
                            else:
                                nc.sync.dma_start(out[:, blk * 128:(blk + 1) * 128], h_t[:])
                        blk_base += nb
                    if layer == 1 and not timing_loop:
                        nc.gpsimd.collective_compute(
                            "AllGather", ALU.bypass, replica_groups=rg,
                            ins=[shard2.opt()], outs=[table2.opt()])
            if timing_loop:
                loop_cm.__exit__(None, None, None)
    nc.compile()
    return nc


def make_runner(nc, n_cores):
    """Build a cached jitted executor for nc (avoids per-call re-jit + NEFF
    reload). Returns run(in_maps) -> list of per-core {name: np.ndarray}."""
    import jax
    import numpy as np
    from jax.sharding import Mesh, PartitionSpec
    from jax.experimental.shard_map import shard_map
    from concourse import bass2jax
    from concourse.bass2jax import _bass_exec_p, partition_id_tensor

    bass2jax.install_neuronx_cc_hook()
    in_names, out_names, out_avals, zero_outs = [], [], [], []
    pname = nc.partition_id_tensor.name if nc.partition_id_tensor else None
    for alloc in nc.m.functions[0].allocations:
        if not isinstance(alloc, mybir.MemoryLocationSet):
            continue
        name = alloc.memorylocations[0].name
        if alloc.kind == "ExternalInput":
            if name != pname:
                in_names.append(name)
        elif alloc.kind == "ExternalOutput":
            shape = tuple(alloc.tensor_shape)
            dtype = mybir.dt.np(alloc.dtype)
            out_names.append(name)
            out_avals.append(jax.core.ShapedArray(shape, dtype))
            zero_outs.append(np.zeros(shape, dtype))
    n_params = len(in_names)
    all_in = list(in_names) + list(out_names)
    if pname is not None:
        all_in.append(pname)

    def _body(*args):
        operands = list(args)
        if pname is not None:
            operands.append(partition_id_tensor())
        outs = _bass_exec_p.bind(
            *operands,
            out_avals=tuple(out_avals),
            in_names=tuple(all_in),
            out_names=tuple(out_names),
            lowering_input_output_aliases=(),
            sim_require_finite=True,
            sim_require_nnan=True,
            nc=nc,
        )
        return tuple(outs)

    donate = tuple(range(n_params, n_params + len(out_avals)))
    if n_cores == 1:
        fn = jax.jit(_body, donate_argnums=donate, keep_unused=True)

        def run(in_maps):
            args = [np.asarray(in_maps[0][n]) for n in in_names]
            outs = fn(*args, *[np.zeros_like(z) for z in zero_outs])
            return [{n: np.asarray(outs[i]) for i, n in enumerate(out_names)}]
        return run

    devices = jax.devices()[:n_cores]
    mesh = Mesh(np.asarray(devices), ("core",))
    in_specs = (PartitionSpec("core"),) * (n_params + len(out_avals))
    out_specs = (PartitionSpec("core"),) * len(out_names)
    fn = jax.jit(
        shard_map(_body, mesh=mesh, in_specs=in_specs, out_specs=out_specs,
                  check_rep=False),
        donate_argnums=donate, keep_unused=True)

    def run(in_maps):
        concat_in = [
            np.concatenate([np.asarray(in_maps[c][n]) for c in range(n_cores)], axis=0)
            for n in in_names]
        concat_zero = [np.zeros((n_cores * z.shape[0], *z.shape[1:]), z.dtype)
                       for z in zero_outs]
        outs = fn(*concat_in, *concat_zero)
        return [
            {n: np.asarray(outs[i]).reshape(n_cores, *out_avals[i].shape)[c]
             for i, n in enumerate(out_names)}
            for c in range(n_cores)]
    return run


def make_in_maps(x, w1, b1, w2, b2, cfg: Cfg, per_core_meta):
    iota = np.tile(np.arange(128, dtype=np.float32), (128, 1))
    xpad = np.zeros((cfg.n_pad, cfg.in_dim), dtype=np.float32)
    xpad[:cfg.n_nodes] = x
    maps = []
    for c in range(cfg.n_cores):
        xs = xpad[c * cfg.shard:(c + 1) * cfg.shard]
        maps.append({
            "x_t": np.ascontiguousarray(xs.T),
            "w1": np.asarray(w1, np.float32),
            "w2": np.asarray(w2, np.float32),
            "b1": np.asarray(b1, np.float32).reshape(-1, 1),
            "b2": np.asarray(b2, np.float32).reshape(-1, 1),
            "iota": iota,
            "idx": per_core_meta[c]["idx"],
            "dest": per_core_meta[c]["dest"],
            "norm": per_core_meta[c]["norm"],
        })
    return maps


def assemble_output(results, cfg: Cfg):
    outs = [np.asarray(r["out"]).T for r in results]  # [shard, hid] each
    full = np.concatenate(outs, axis=0)
    return full[:cfg.n_nodes]


# ---------------------------------------------------------------------------
# Harness entry point
# ---------------------------------------------------------------------------
_CACHE = {}

N_NODES = 100000
IN_DIM = 256
HID_DIM = 128
N_CORES = 8


def kernel(x, edge_index, W1, b1, W2, b2):
    x = np.asarray(x, dtype=np.float32)
    edge_index = np.asarray(edge_index)
    cfg = Cfg(N_NODES, IN_DIM, HID_DIM, n_cores=N_CORES, blocks_per_sb=4,
              chunk_rows=25088)
    per_core, Bbc, sb_sizes = preprocess(edge_index, cfg)
    key = (bytes(np.asarray(Bbc)), tuple(sb_sizes))
    if key not in _CACHE:
        nc = build_kernel(cfg, Bbc, sb_sizes, nqueues=4)
        run = make_runner(nc, N_CORES)
        _CACHE[key] = run
    run = _CACHE[key]
    maps = make_in_maps(x, W1, b1, W2, b2, cfg, per_core)
    results = run(maps)
    return np.ascontiguousarray(assemble_output(results, cfg).astype(np.float32))



# revision 2
# speedup vs baseline: 10.2948x; 10.2948x over previous
"""2-layer GCN on 8 Trainium2 cores via Bass/Tile — v3.

v2 -> v3:
- dynamic_dma_scratch_size raised 16K->64K so a full (sb, chunk) gather's
  descriptors fit in the SWDGE ring; the Pool engine no longer babysits
  each gather, letting the 4 queues' DMAs overlap.
- Segment idx/S loads batched to one DMA per super-block (contiguous in
  DRAM); self-rows and output writes batched per super-block as well.
- Load DMAs issued from the Activation queue, writes from SP, spreading
  HWDGE sequencer work.
- AllGather outputs (table1/2) allocated in the Shared DRAM scratchpad.
- Bias rank-1 matmuls skipped when the bias vector is all zero.
"""
import numpy as np
import ml_dtypes
import concourse.bass as bass
import concourse.bacc as bacc
import concourse.tile as tile
import concourse.mybir as mybir

F32 = mybir.dt.float32
BF16 = mybir.dt.bfloat16
FP8 = mybir.dt.float8e4
I16 = mybir.dt.int16
AF = mybir.ActivationFunctionType
ALU = mybir.AluOpType

NP_BF16 = ml_dtypes.bfloat16
NP_FP8 = ml_dtypes.float8_e4m3


class Cfg:
    def __init__(self, n_nodes, in_dim, hid, n_cores=8, blocks_per_sb=4,
                 chunk_rows=25088):
        self.n_nodes = n_nodes
        self.in_dim = in_dim
        self.hid = hid
        self.n_cores = n_cores
        self.bpsb = blocks_per_sb
        shard = -(-n_nodes // n_cores)
        shard = -(-shard // 128) * 128
        self.shard = shard
        self.n_pad = shard * n_cores
        self.blocks = shard // 128
        self.chunk_rows = chunk_rows
        assert self.n_pad % chunk_rows == 0
        self.n_chunks = self.n_pad // chunk_rows
        assert chunk_rows <= 32767
        self.n_sb = -(-self.blocks // self.bpsb)
        self.row_bytes = hid * 4


def preprocess(edge_index, cfg: Cfg):
    """Bucket, pad, and lay out edges (self-loops handled separately)."""
    n = cfg.n_nodes
    row = np.asarray(edge_index[0], dtype=np.int64)
    col = np.asarray(edge_index[1], dtype=np.int64)
    deg = np.bincount(col, minlength=n).astype(np.float64) + 1.0
    dinv = (1.0 / np.sqrt(deg)).astype(np.float32)
    dinv_pad = np.zeros(cfg.n_pad, dtype=np.float32)
    dinv_pad[:n] = dinv

    core = col // cfg.shard
    block = (col % cfg.shard) // 128
    dest_local = (col % 128).astype(np.int64)
    chunk = row // cfg.chunk_rows
    src_rel = (row % cfg.chunk_rows).astype(np.int16)

    nbuck = cfg.n_cores * cfg.blocks * cfg.n_chunks
    bid = (core * cfg.blocks + block) * cfg.n_chunks + chunk
    order = np.argsort(bid, kind="stable")
    counts = np.bincount(bid, minlength=nbuck)
    cmax = counts.reshape(cfg.n_cores, cfg.blocks, cfg.n_chunks).max(axis=0)
    Bbc = (-(-cmax // 128) * 128).astype(np.int64)  # [blocks, n_chunks]

    starts = np.zeros(nbuck + 1, dtype=np.int64)
    np.cumsum(counts, out=starts[1:])
    src_rel_s = src_rel[order]
    dest_local_s = dest_local[order]

    sb_sizes = []
    bucket_off = np.zeros((cfg.blocks, cfg.n_chunks), dtype=np.int64)
    _o = 0
    for sb in range(cfg.n_sb):
        b0 = sb * cfg.bpsb
        nb = min(cfg.bpsb, cfg.blocks - b0)
        sb_sizes.append(nb)
        for ch in range(cfg.n_chunks):
            for b in range(b0, b0 + nb):
                bucket_off[b, ch] = _o
                _o += Bbc[b, ch]
    n_idx_tot = _o
    assert n_idx_tot == int(Bbc.sum())

    per_core = []
    for c in range(cfg.n_cores):
        idx_flat = np.zeros(n_idx_tot, dtype=np.int16)
        pp_parts, cc_parts = [], []
        for b in range(cfg.blocks):
            for ch in range(cfg.n_chunks):
                k = (c * cfg.blocks + b) * cfg.n_chunks + ch
                s, e = starts[k], starts[k + 1]
                cnt = e - s
                if cnt == 0:
                    continue
                j = bucket_off[b, ch] + np.arange(cnt, dtype=np.int64)
                idx_flat[j] = src_rel_s[s:e]
                pp_parts.append(j % 128)
                cc_parts.append((j // 128) * 128 + dest_local_s[s:e])
        S = np.zeros((128, n_idx_tot), dtype=NP_FP8)
        if pp_parts:
            S[np.concatenate(pp_parts), np.concatenate(cc_parts)] = 1.0
        wrapped = np.tile(idx_flat.reshape(-1, 16).T, (8, 1))  # [128, ntot/16]
        dv = dinv_pad[c * cfg.shard:(c + 1) * cfg.shard]
        dinv_t = np.ascontiguousarray(dv.reshape(cfg.blocks, 128).T)  # [128, blocks]
        invd = np.where(dv > 0, 1.0 / np.maximum(dv, 1e-30), 0.0)
        invd = invd.astype(NP_BF16).reshape(1, cfg.shard)
        per_core.append(dict(idx=wrapped, S=S, dinv=dinv_t, invd=invd))
    return per_core, Bbc, sb_sizes


def build_kernel(cfg: Cfg, Bbc, sb_sizes, nqueues=4, repeat=1, timing_loop=0,
                 use_bias=(True, True), dma_scratch=65536, shared_tables=True,
                 single_packet=False):
    """Build the SPMD Bass program (identical for all cores)."""
    hid, ind = cfg.hid, cfg.in_dim
    Bbc = np.asarray(Bbc, dtype=np.int64)
    n_idx_tot = int(Bbc.sum())
    kparts = ind // 128
    nc = bacc.Bacc("TRN2", target_bir_lowering=False, debug=False,
                   num_devices=cfg.n_cores, num_swdge_queues=nqueues,
                   dynamic_dma_scratch_size=dma_scratch)

    # ---- I/O ----
    x_t = nc.dram_tensor("x_t", [ind, cfg.shard], BF16, kind="ExternalInput")
    w1 = nc.dram_tensor("w1", [ind, hid], BF16, kind="ExternalInput")
    w2 = nc.dram_tensor("w2", [hid, hid], BF16, kind="ExternalInput")
    b1r = nc.dram_tensor("b1r", [1, hid], BF16, kind="ExternalInput")
    b2r = nc.dram_tensor("b2r", [1, hid], BF16, kind="ExternalInput")
    invd_in = nc.dram_tensor("invd", [1, cfg.shard], BF16, kind="ExternalInput")
    dinv_in = nc.dram_tensor("dinv", [128, cfg.blocks], F32, kind="ExternalInput")
    id8_in = nc.dram_tensor("ident8", [128, 128], FP8, kind="ExternalInput")
    id16_in = nc.dram_tensor("ident16", [128, 128], BF16, kind="ExternalInput")
    idx_in = nc.dram_tensor("idx", [128, n_idx_tot // 16], I16, kind="ExternalInput")
    s_in = nc.dram_tensor("s", [128, n_idx_tot], FP8, kind="ExternalInput")
    out = nc.dram_tensor("out", [cfg.shard, hid], F32, kind="ExternalOutput")

    rg = [list(range(cfg.n_cores))]
    taddr = "Shared" if shared_tables else "Local"

    with tile.TileContext(nc) as tc:
        with (
            tc.tile_pool(name="dram", bufs=1, space="DRAM") as dram,
            tc.tile_pool(name="const", bufs=1) as cpool,
            tc.tile_pool(name="xin", bufs=4) as xpool,
            tc.tile_pool(name="mmps", bufs=2, space="PSUM") as mmps,
            tc.tile_pool(name="aggps", bufs=3, space="PSUM") as aggps,
            tc.tile_pool(name="tpps", bufs=1, space="PSUM") as tpps,
            tc.tile_pool(name="gat", bufs=2 * cfg.n_chunks) as gatpool,
            tc.tile_pool(name="sseg", bufs=3) as spool,
            tc.tile_pool(name="iseg", bufs=3) as ipool,
            tc.tile_pool(name="selfp", bufs=3) as selfpool,
            tc.tile_pool(name="hp", bufs=6) as hpool,
            tc.tile_pool(name="wr", bufs=3) as wrpool,
        ):
            shard1 = dram.tile([cfg.shard, hid], BF16)
            shard2 = dram.tile([cfg.shard, hid], BF16)
            table1 = dram.tile([cfg.n_pad, hid], BF16, addr_space=taddr)
            table2 = dram.tile([cfg.n_pad, hid], BF16, addr_space=taddr)

            # constants
            w1_t = cpool.tile([128, kparts, hid], BF16, tag="w1")
            nc.sync.dma_start(w1_t[:], w1[:].rearrange("(k p) h -> p k h", p=128))
            w2_t = cpool.tile([128, hid], BF16, tag="w2")
            nc.sync.dma_start(w2_t[:], w2[:])
            b1_t = cpool.tile([1, hid], BF16, tag="b1")
            nc.sync.dma_start(b1_t[:], b1r[:])
            b2_t = cpool.tile([1, hid], BF16, tag="b2")
            nc.sync.dma_start(b2_t[:], b2r[:])
            invd_t = cpool.tile([1, cfg.shard], BF16, tag="invd")
            nc.sync.dma_start(invd_t[:], invd_in[:])
            dinv_t = cpool.tile([128, cfg.blocks], F32, tag="dinv")
            nc.sync.dma_start(dinv_t[:], dinv_in[:])
            id8_t = cpool.tile([128, 128], FP8, tag="id8")
            nc.sync.dma_start(id8_t[:], id8_in[:])
            id16_t = cpool.tile([128, 128], BF16, tag="id16")
            nc.sync.dma_start(id16_t[:], id16_in[:])

            # segment offsets: layout is for sb: for ch: for b in sb
            seg_off, seg_len = {}, {}
            _o = 0
            for sb, nb in enumerate(sb_sizes):
                b0 = sb * cfg.bpsb
                for ch in range(cfg.n_chunks):
                    seg_off[(sb, ch)] = _o
                    L = int(Bbc[b0:b0 + nb, ch].sum())
                    seg_len[(sb, ch)] = L
                    _o += L
            assert _o == n_idx_tot

            if timing_loop:
                nc.gpsimd.collective_compute(
                    "AllGather", ALU.bypass, replica_groups=rg,
                    ins=[shard1.opt()], outs=[table1.opt()])
                nc.gpsimd.collective_compute(
                    "AllGather", ALU.bypass, replica_groups=rg,
                    ins=[shard2.opt()], outs=[table2.opt()])
                loop_cm = tc.For_i(0, timing_loop, 1)
                loop_cm.__enter__()

            for _rep in range(repeat):
                # ---- prologue: shard1 = dinv * (x @ W1), bf16 ----
                blk_base = 0
                for sb, nb in enumerate(sb_sizes):
                    b0 = sb * cfg.bpsb
                    sb_ = wrpool.tile([128, cfg.bpsb, hid], BF16, tag="s1")
                    for b in range(nb):
                        nblk = blk_base + b
                        xt = xpool.tile([128, kparts, 128], BF16)
                        nc.scalar.dma_start(
                            xt[:], x_t[:, nblk * 128:(nblk + 1) * 128]
                            .rearrange("(k p) d -> p k d", p=128))
                        ps = mmps.tile([128, hid], F32, tag="ps")
                        for k in range(kparts):
                            nc.tensor.matmul(ps[:], xt[:, k, :], w1_t[:, k, :],
                                             start=(k == 0), stop=(k == kparts - 1))
                        nc.vector.tensor_scalar_mul(sb_[:, b, :], ps[:],
                                                    dinv_t[:, nblk:nblk + 1])
                    nc.sync.dma_start(
                        shard1[blk_base * 128:(blk_base + nb) * 128, :]
                        .rearrange("(g p) f -> p g f", p=128), sb_[:, :nb, :])
                    blk_base += nb

                if not timing_loop:
                    nc.gpsimd.collective_compute(
                        "AllGather", ALU.bypass, replica_groups=rg,
                        ins=[shard1.opt()], outs=[table1.opt()])

                # ---- layers ----
                for layer in (1, 2):
                    table = table1 if layer == 1 else table2
                    shard_l = shard1 if layer == 1 else shard2
                    brow = b1_t if layer == 1 else b2_t
                    add_bias = use_bias[layer - 1]
                    blk_base = 0
                    for sb, nb in enumerate(sb_sizes):
                        b0 = sb * cfg.bpsb
                        o_sb = seg_off[(sb, 0)]
                        L_sb = sum(seg_len[(sb, ch)] for ch in range(cfg.n_chunks))
                        # batched idx + S loads for the whole super-block
                        it = ipool.tile([128, L_sb // 16], I16, tag="it")
                        nc.scalar.dma_start(
                            it[:], idx_in[:, o_sb // 16:(o_sb + L_sb) // 16])
                        st = spool.tile([128, L_sb], FP8, tag="st")
                        nc.scalar.dma_start(st[:], s_in[:, o_sb:o_sb + L_sb])
                        # batched self rows for the super-block
                        selft = selfpool.tile([128, cfg.bpsb, hid], BF16)
                        nc.scalar.dma_start(
                            selft[:, :nb, :],
                            shard_l[blk_base * 128:(blk_base + nb) * 128, :]
                            .rearrange("(g p) f -> p g f", p=128))
                        gts = []
                        for ch in range(cfg.n_chunks):
                            o = seg_off[(sb, ch)]
                            L = seg_len[(sb, ch)]
                            if L == 0:
                                gts.append(None)
                                continue
                            gt = gatpool.tile([128, L // 128, hid], BF16, tag="gt")
                            nc.gpsimd.dma_gather(
                                gt[:],
                                table[ch * cfg.chunk_rows:(ch + 1) * cfg.chunk_rows, :],
                                it[:, (o - o_sb) // 16:(o - o_sb + L) // 16], L, L, hid,
                                single_packet=single_packet, queue_num=ch % nqueues)
                            gts.append(gt)
                        if layer == 1:
                            st2g = wrpool.tile([128, cfg.bpsb, hid], BF16, tag="st2")
                        else:
                            o_tg = hpool.tile([128, cfg.bpsb, hid], F32, tag="o")
                        for b in range(nb):
                            blk = blk_base + b
                            agg = aggps.tile([128, 128], F32, tag="agg")
                            mm_i = 0
                            for ch, gt in enumerate(gts):
                                if gt is None:
                                    continue
                                o = seg_off[(sb, ch)]
                                gpb = int(Bbc[b0 + b, ch]) // 128
                                goff = (o - o_sb + int(Bbc[b0:b0 + b, ch].sum())) // 128
                                for k in range(gpb):
                                    g = goff + k
                                    gl = g - (o - o_sb) // 128
                                    nc.tensor.matmul(
                                        agg[:], st[:, g * 128:(g + 1) * 128],
                                        gt[:, gl, :],
                                        start=(mm_i == 0), stop=False)
                                    mm_i += 1
                            # self-loop contribution
                            is_last = not add_bias
                            nc.tensor.matmul(agg[:], id8_t[:], selft[:, b, :],
                                             start=(mm_i == 0), stop=is_last)
                            mm_i += 1
                            if add_bias:
                                nc.tensor.matmul(
                                    agg[:], invd_t[0:1, blk * 128:(blk + 1) * 128],
                                    brow[:], start=False, stop=True)
                            if layer == 1:
                                h1 = hpool.tile([128, 128], BF16, tag="h1")
                                nc.scalar.activation(h1[:], agg[:], AF.Tanh,
                                                     scale=dinv_t[:, blk:blk + 1])
                                tp = tpps.tile([128, 128], BF16)
                                nc.tensor.transpose(tp[:], h1[:], id16_t[:])
                                h1T = hpool.tile([128, 128], BF16, tag="h1T")
                                nc.vector.tensor_copy(h1T[:], tp[:])
                                ps2 = mmps.tile([128, hid], F32, tag="ps2")
                                nc.tensor.matmul(ps2[:], h1T[:], w2_t[:],
                                                 start=True, stop=True)
                                nc.vector.tensor_scalar_mul(
                                    st2g[:, b, :], ps2[:], dinv_t[:, blk:blk + 1])
                            else:
                                nc.scalar.activation(o_tg[:, b, :], agg[:], AF.Tanh,
                                                     scale=dinv_t[:, blk:blk + 1])
                        if layer == 1:
                            nc.sync.dma_start(
                                shard2[blk_base * 128:(blk_base + nb) * 128, :]
                                .rearrange("(g p) f -> p g f", p=128), st2g[:, :nb, :])
                        else:
                            nc.sync.dma_start(
                                out[blk_base * 128:(blk_base + nb) * 128, :]
                                .rearrange("(g p) f -> p g f", p=128), o_tg[:, :nb, :])
                        blk_base += nb
                    if layer == 1 and not timing_loop:
                        nc.gpsimd.collective_compute(
                            "AllGather", ALU.bypass, replica_groups=rg,
                            ins=[shard2.opt()], outs=[table2.opt()])
            if timing_loop:
                loop_cm.__exit__(None, None, None)
    nc.compile()
    return nc


def make_runner(nc, n_cores):
    """Build a cached jitted executor for nc."""
    import jax
    import numpy as np
    from jax.sharding import Mesh, PartitionSpec
    from jax.experimental.shard_map import shard_map
    from concourse import bass2jax
    from concourse.bass2jax import _bass_exec_p, partition_id_tensor

    bass2jax.install_neuronx_cc_hook()
    in_names, out_names, out_avals, zero_outs = [], [], [], []
    pname = nc.partition_id_tensor.name if nc.partition_id_tensor else None
    for alloc in nc.m.functions[0].allocations:
        if not isinstance(alloc, mybir.MemoryLocationSet):
            continue
        name = alloc.memorylocations[0].name
        if alloc.kind == "ExternalInput":
            if name != pname:
                in_names.append(name)
        elif alloc.kind == "ExternalOutput":
            shape = tuple(alloc.tensor_shape)
            dtype = mybir.dt.np(alloc.dtype)
            out_names.append(name)
            out_avals.append(jax.core.ShapedArray(shape, dtype))
            zero_outs.append(np.zeros(shape, dtype))
    n_params = len(in_names)
    all_in = list(in_names) + list(out_names)
    if pname is not None:
        all_in.append(pname)

    def _body(*args):
        operands = list(args)
        if pname is not None:
            operands.append(partition_id_tensor())
        outs = _bass_exec_p.bind(
            *operands,
            out_avals=tuple(out_avals),
            in_names=tuple(all_in),
            out_names=tuple(out_names),
            lowering_input_output_aliases=(),
            sim_require_finite=True,
            sim_require_nnan=True,
            nc=nc,
        )
        return tuple(outs)

    donate = tuple(range(n_params, n_params + len(out_avals)))
    if n_cores == 1:
        fn = jax.jit(_body, donate_argnums=donate, keep_unused=True)

        def run(in_maps):
            args = [np.asarray(in_maps[0][n]) for n in in_names]
            outs = fn(*args, *[np.zeros_like(z) for z in zero_outs])
            return [{n: np.asarray(outs[i]) for i, n in enumerate(out_names)}]
        return run

    devices = jax.devices()[:n_cores]
    mesh = Mesh(np.asarray(devices), ("core",))
    in_specs = (PartitionSpec("core"),) * (n_params + len(out_avals))
    out_specs = (PartitionSpec("core"),) * len(out_names)
    fn = jax.jit(
        shard_map(_body, mesh=mesh, in_specs=in_specs, out_specs=out_specs,
                  check_rep=False),
        donate_argnums=donate, keep_unused=True)

    def run(in_maps):
        concat_in = [
            np.concatenate([np.asarray(in_maps[c][n]) for c in range(n_cores)], axis=0)
            for n in in_names]
        concat_zero = [np.zeros((n_cores * z.shape[0], *z.shape[1:]), z.dtype)
                       for z in zero_outs]
        outs = fn(*concat_in, *concat_zero)
        return [
            {n: np.asarray(outs[i]).reshape(n_cores, *out_avals[i].shape)[c]
             for i, n in enumerate(out_names)}
            for c in range(n_cores)]
    return run


def make_in_maps(x, w1, b1, w2, b2, cfg: Cfg, per_core_meta):
    xpad = np.zeros((cfg.n_pad, cfg.in_dim), dtype=np.float32)
    xpad[:cfg.n_nodes] = np.asarray(x, np.float32)
    w1b = np.asarray(w1, np.float32).astype(NP_BF16)
    w2b = np.asarray(w2, np.float32).astype(NP_BF16)
    b1b = np.asarray(b1, np.float32).astype(NP_BF16).reshape(1, -1)
    b2b = np.asarray(b2, np.float32).astype(NP_BF16).reshape(1, -1)
    id8 = np.eye(128, dtype=NP_FP8)
    id16 = np.eye(128, dtype=NP_BF16)
    maps = []
    for c in range(cfg.n_cores):
        xs = xpad[c * cfg.shard:(c + 1) * cfg.shard]
        m = per_core_meta[c]
        maps.append({
            "x_t": np.ascontiguousarray(xs.T).astype(NP_BF16),
            "w1": w1b, "w2": w2b, "b1r": b1b, "b2r": b2b,
            "invd": m["invd"], "dinv": m["dinv"],
            "ident8": id8, "ident16": id16,
            "idx": m["idx"], "s": m["S"],
        })
    return maps


def assemble_output(results, cfg: Cfg):
    outs = [np.asarray(r["out"]) for r in results]  # [shard, hid] node-major
    full = np.concatenate(outs, axis=0)
    return full[:cfg.n_nodes]


# ---------------------------------------------------------------------------
# Harness entry point
# ---------------------------------------------------------------------------
_CACHE = {}

N_NODES = 100000
IN_DIM = 256
HID_DIM = 128
N_CORES = 8


def kernel(x, edge_index, W1, b1, W2, b2):
    x = np.asarray(x, dtype=np.float32)
    edge_index = np.asarray(edge_index)
    cfg = Cfg(N_NODES, IN_DIM, HID_DIM, n_cores=N_CORES, blocks_per_sb=4,
              chunk_rows=25088)
    per_core, Bbc, sb_sizes = preprocess(edge_index, cfg)
    use_bias = (bool(np.any(np.asarray(b1))), bool(np.any(np.asarray(b2))))
    key = (bytes(np.asarray(Bbc)), tuple(sb_sizes), use_bias)
    if key not in _CACHE:
        nc = build_kernel(cfg, Bbc, sb_sizes, nqueues=4, use_bias=use_bias)
        run = make_runner(nc, N_CORES)
        _CACHE[key] = run
    run = _CACHE[key]
    maps = make_in_maps(x, W1, b1, W2, b2, cfg, per_core)
    results = run(maps)
    return np.ascontiguousarray(assemble_output(results, cfg).astype(np.float32))
